# revision 14
# baseline (speedup 1.0000x reference)
"""Trainium2 Bass kernel for nn_Enhanced_transformer (dense transformer block).

Strategy (v3)
-------------
Data-parallel: one batch element per NeuronCore (8 cores), no collectives.
Channel-major layout [channel-part, token-free]. Value path restructured as
W_eff = att^T v_w so only one [N,P,P] apply GEMM remains; LN1 folded via
xs = x*rho (pre-scaled moving operand) plus wsum/bsum corrections in the x1
epilogue; LN2 stats via fp8-DoubleRow ones-matmuls.

dtype plan (error budget: gate 2e-2, sim ~1.6e-2):
 - xq/energy/att-logits path: f32r (precision-critical, x read as f32r in A)
 - apply GEMM + m1 GEMM + weff build: bf16 (1 cyc/row, FWL weight loads)
 - m2 GEMM: fp8e4m3 DoubleRow (2x): mg pairs from ACT gelu directly, m2
   weights host-quantized at x64; LN2-stat GEMMs fp8-DR with x1/sq pair
   copies from ACT. m2_b is added host-side.
 - x uploaded twice: f32r (phase A) and bf16 (phase C moving/residual).

Per-chunk phase C emit order keeps PE dense: apply MMs -> prev-chunk MLP
MMs -> stats -> h2; MLP runs one chunk behind.
"""

import ml_dtypes
import numpy as np

import concourse.bass as bass
import concourse.tile as tile
from concourse import bacc, mybir
from concourse import bass_utils



F32 = mybir.dt.float32
F32R = mybir.dt.float32r
BF16 = mybir.dt.bfloat16
FP8 = mybir.dt.float8e4
AF = mybir.ActivationFunctionType
ALU = mybir.AluOpType
AX = mybir.AxisListType
DR = mybir.MatmulPerfMode.DoubleRow

B, N, P = 8, 4096, 1024
P4 = P // 4          # 256
EPS = 1e-5
CH = 512             # token chunk
NCH = N // CH        # 8
KP = P // 128        # 8 channel tiles
KP2 = KP // 2        # 4 channel pair tiles
TS = CH // 128       # 4 token sub-tiles per chunk
INV_P = 1.0 / P
S_W = 64.0           # fp8 weight scale (m2)
ONES8 = 2.0 ** -4    # fp8 ones value for stat matmuls
C_STAT = 2.0 ** -6   # ONES8 * P / P ... psum = 2^-4 * 1024 * mean = 64*mean
C_M2 = 2.0 ** -6     # m2 psum descale (weights x64)


def _build(loop_R: int = 1, loop_phase: str = 'all'):
    nc = bacc.Bacc("TRN2", target_bir_lowering=False, debug=False)

    # ---- DRAM I/O ----
    xT_d = nc.dram_tensor("xT", [P, N], F32R, kind="ExternalInput").ap()
    xTb_d = nc.dram_tensor("xTb", [P, N], BF16, kind="ExternalInput").ap()
    qk_wT_d = nc.dram_tensor("qk_wT", [P, P4 + 2], F32R,
                             kind="ExternalInput").ap()
    qksum_d = nc.dram_tensor("qksum", [P4], F32, kind="ExternalInput").ap()
    qkb_d = nc.dram_tensor("qkb", [P4], F32, kind="ExternalInput").ap()
    v_w_d = nc.dram_tensor("v_w", [P, P], BF16, kind="ExternalInput").ap()
    vbr_d = nc.dram_tensor("vbr", [P, 2], BF16, kind="ExternalInput").ap()
    t1_wT_d = nc.dram_tensor("t1_wT", [P4, P], F32R, kind="ExternalInput").ap()
    t1_b_d = nc.dram_tensor("t1_b", [P], F32, kind="ExternalInput").ap()
    t2_wT_d = nc.dram_tensor("t2_wT", [P4, P], F32R, kind="ExternalInput").ap()
    m1_wT_d = nc.dram_tensor("m1_wT", [P, P], BF16, kind="ExternalInput").ap()
    m1_b_d = nc.dram_tensor("m1_b", [P], F32, kind="ExternalInput").ap()
    m2p_d = nc.dram_tensor("m2p", [KP2 * 128, 2 * P], FP8,
                           kind="ExternalInput").ap()
    ones8_d = nc.dram_tensor("ones8", [128, 256], FP8,
                             kind="ExternalInput").ap()
    iden_d = nc.dram_tensor("iden", [128, 128], F32,
                            kind="ExternalInput").ap()
    outT_d = nc.dram_tensor("outT", [P, N], F32, kind="ExternalOutput").ap()

    def bcast_src(vec_ap):
        """[n] dram vector -> stride-0 partition-broadcast src AP [128, n]."""
        return bass.AP(tensor=vec_ap.tensor, offset=vec_ap.offset,
                       ap=[[0, 128], *vec_ap.ap])

    def part_bias_tiles(pool, dram_ap, name, dtype=F32):
        tiles = []
        for t in range(KP):
            bt = pool.tile([128, 1], dtype, tag=f"{name}{t}", name=f"{name}{t}")
            nc.gpsimd.dma_start(bt[:], dram_ap[t * 128:(t + 1) * 128])
            tiles.append(bt)
        return tiles

    with tile.TileContext(nc) as tc:
        with (
            nc.allow_low_precision(reason="bf16/fp8 epilogues validated in sim"),
            tc.tile_pool(name="dram", bufs=1, space="DRAM") as dram_pool,
            tc.tile_pool(name="consts", bufs=1) as consts,
        ):

            rho_row_d = dram_pool.tile([N], F32, name="rho_row")
            nmr_row_d = dram_pool.tile([N], F32, name="nmr_row")

            # ---- persistent constants (loaded once) ----
            ones_f = consts.tile([128, 128], F32, tag="ones_f", name="ones_f")
            nc.vector.memset(ones_f[:], INV_P)
            ones_r = consts.tile([128, 128], F32R, tag="ones_r", name="ones_r")
            nc.vector.tensor_copy(ones_r[:], ones_f[:])
            eps_t = consts.tile([128, 1], F32, tag="eps", name="eps_t")
            nc.vector.memset(eps_t[:], EPS)
            cm2_t = consts.tile([128, 1], F32, tag="cm2", name="cm2_t")
            nc.vector.memset(cm2_t[:], C_M2)
            ones8 = consts.tile([128, 2, 128], FP8, tag="ones8", name="ones8")
            nc.scalar.dma_start(ones8[:], ones8_d)
            iden = consts.tile([128, 128], F32, tag="iden", name="iden")
            nc.scalar.dma_start(iden[:], iden_d)
            m1b_t = part_bias_tiles(consts, m1_b_d, "m1b")

            from contextlib import ExitStack as _ES
            _loop_ctx = _ES()
            if loop_R > 1 and loop_phase == 'all':
                _loop_ctx.enter_context(tc.For_i(0, loop_R, 1))

            def phase_loop(tag):
                es = _ES()
                if loop_R > 1 and loop_phase == tag:
                    es.enter_context(tc.For_i(0, loop_R, 1))
                return es

            with tc.tile_pool(name="wBC", bufs=1) as wBC:  # survives into C
                bsum_t = [wBC.tile([128, 1], F32, tag=f"bs{q}", name=f"bs{q}")
                          for q in range(KP)]
                wsum_t = [wBC.tile([128, 1], F32, tag=f"ws{q}", name=f"ws{q}")
                          for q in range(KP)]
                rho_t = wBC.tile([128, TS * NCH], F32, tag="rho_t",
                                 name="rho_t")
                nmr_t = wBC.tile([128, TS * NCH], F32, tag="nmr_t",
                                 name="nmr_t")
                mu_t = wBC.tile([128, TS * NCH], F32, tag="mu_t", name="mu_t")
                rhoT = wBC.tile([32, 128], F32, tag="rhoT", name="rhoT")
                nmrT = wBC.tile([32, 128], F32, tag="nmrT", name="nmrT")
                weff = [wBC.tile([128, P], BF16, tag=f"wf{c}", name=f"wf{c}")
                        for c in range(KP)]

                with tc.tile_pool(name="wB", bufs=1) as wB:  # closed after B
                    # ---- weights (per iteration, overlap with phase A) ----
                    qk_ext = []
                    for p in range(KP):
                        t = wB.tile([128, P4 + 2], F32R, tag=f"qke{p}",
                                    name=f"qke{p}")
                        eng = nc.sync if p % 2 == 0 else nc.scalar
                        eng.dma_start(t[:], qk_wT_d[p * 128:(p + 1) * 128, :])
                        qk_ext.append(t)
                    qksum_bc = wB.tile([128, P4], F32, tag="qksbc",
                                       name="qksbc")
                    nc.sync.dma_start(qksum_bc[:], bcast_src(qksum_d))
                    qkb_bc = wB.tile([128, P4], F32, tag="qkbbc", name="qkbbc")
                    nc.sync.dma_start(qkb_bc[:], bcast_src(qkb_d))
                    wload = []    # issued during phase A (ACT queue)
                    wload_b = []  # issued during phase B (ACT queue)
                    v_w_r, t1w, t2w = [], [], []
                    for p in range(KP):
                        t = wB.tile([128, P], BF16, tag=f"vw{p}", name=f"vw{p}")
                        wload_b.append((t[:], v_w_d[p * 128:(p + 1) * 128, :]))
                        v_w_r.append(t)
                    for i in range(2):
                        t = wB.tile([128, P], F32R, tag=f"t1w{i}",
                                    name=f"t1w{i}")
                        wload.append((t[:], t1_wT_d[i * 128:(i + 1) * 128, :]))
                        t1w.append(t)
                        t2 = wB.tile([128, P], F32R, tag=f"t2w{i}",
                                     name=f"t2w{i}")
                        wload.append((t2[:], t2_wT_d[i * 128:(i + 1) * 128, :]))
                        t2w.append(t2)
                    t1b_bc = wB.tile([128, P], F32, tag="t1bbc", name="t1bbc")
                    wload.append((t1b_bc[:], bcast_src(t1_b_d)))
                    vb_r = []
                    for p in range(KP):
                        t = wB.tile([128, 2], BF16, tag=f"vbr{p}",
                                    name=f"vbr{p}")
                        wload.append((t[:], vbr_d[p * 128:(p + 1) * 128, :]))
                        vb_r.append(t)
                    m1w, m2p = [], []
                    for p in range(KP):
                        t = wBC.tile([128, P], BF16, tag=f"m1w{p}",
                                     name=f"m1w{p}")
                        wload_b.append((t[:],
                                        m1_wT_d[p * 128:(p + 1) * 128, :]))
                        m1w.append(t)
                    for p in range(KP2):
                        t = wBC.tile([128, 2, P], FP8, tag=f"m2p{p}",
                                     name=f"m2p{p}")
                        wload_b.append((t[:],
                                        m2p_d[p * 128:(p + 1) * 128, :]))
                        m2p.append(t)

                    with tc.tile_pool(name="psE", bufs=1,
                                      space="PSUM") as psE:
                        e_ps = [psE.tile([128, P4], F32, tag=f"e{i}",
                                         name=f"e_ps{i}")
                                for i in range(2)]

                        # ============ PHASE A ============
                        _pl = phase_loop('A')
                        _phase_a(
                            nc, tc, bass, xT_d, qk_ext, qksum_bc, qkb_bc,
                            ones_r, eps_t, iden, rho_t, nmr_t, mu_t,
                            rhoT, nmrT, rho_row_d, nmr_row_d, e_ps, wload)

                        _pl.close()
                        # energy -> SBUF, then release PSUM banks
                        e_sb = []
                        for i in range(2):
                            t = wBC.tile([128, P4], F32R, tag=f"esb{i}",
                                         name=f"esb{i}")
                            nc.vector.tensor_copy(t[:], e_ps[i][:])
                            e_sb.append(t)

                    # ============ PHASE B ============
                    with (
                        tc.tile_pool(name="pB", bufs=1) as pB,
                        tc.tile_pool(name="psB", bufs=1,
                                     space="PSUM") as psB,
                        phase_loop('B'),
                    ):
                        for dst, src in wload_b:
                            nc.scalar.dma_start(dst, src)
                        G_T = []
                        for jc in range(2):
                            g = pB.tile([128, P], F32R, tag=f"gt{jc}",
                                        name=f"gt{jc}")
                            for kc in range(2):
                                ks = slice(kc * 512, (kc + 1) * 512)
                                ps = psB.tile([128, 512], F32, tag="a1",
                                              name="a1_ps", bufs=1)
                                for qi in range(2):
                                    nc.tensor.matmul(
                                        ps[:],
                                        e_sb[qi][
                                            :, jc * 128:(jc + 1) * 128],
                                        t1w[qi][:, ks],
                                        start=(qi == 0), stop=(qi == 1),
                                    )
                                tmp = pB.tile([128, 512], F32, tag="a1t",
                                              name="a1t", bufs=2)
                                nc.vector.tensor_tensor(
                                    tmp[:], ps[:], t1b_bc[:, ks], ALU.add)
                                nc.scalar.activation(g[:, ks], tmp[:],
                                                     AF.Gelu)
                            G_T.append(g)
                        # t2_b is constant along the softmax axis -> softmax is
                        # invariant to it; skip the bias, run max/exp off PSUM halves.
                        att_r = []
                        for o in range(KP):
                            psh, nmh = [], []
                            for kc in range(2):
                                ks = slice(kc * 512, (kc + 1) * 512)
                                ps = psB.tile([128, 512], F32, tag="a2",
                                              name="a2_ps", bufs=3)
                                for ji in range(2):
                                    nc.tensor.matmul(
                                        ps[:],
                                        t2w[ji][:, o * 128:(o + 1) * 128],
                                        G_T[ji][:, ks],
                                        start=(ji == 0), stop=(ji == 1),
                                    )
                                nm = pB.tile([128, 1], F32, tag=f"nmh{kc}",
                                             name=f"nmh{kc}", bufs=2)
                                nc.vector.tensor_reduce(
                                    nm[:], ps[:], axis=AX.X, op=ALU.max, negate=True)
                                psh.append(ps)
                                nmh.append(nm)
                            negmax = pB.tile([128, 1], F32, tag="negmax",
                                             name="negmax", bufs=2)
                            nc.vector.tensor_tensor(negmax[:], nmh[0][:], nmh[1][:],
                                                    ALU.min)
                            expv = pB.tile([128, P], F32, tag="expv", name="expv",
                                           bufs=2)
                            esh = []
                            for kc in range(2):
                                ks = slice(kc * 512, (kc + 1) * 512)
                                es = pB.tile([128, 1], F32, tag=f"esh{kc}",
                                             name=f"esh{kc}", bufs=2)
                                nc.scalar.activation(
                                    expv[:, ks], psh[kc][:], AF.Exp,
                                    bias=negmax[:], accum_out=es[:])
                                esh.append(es)
                            esum = pB.tile([128, 1], F32, tag="esum", name="esum",
                                           bufs=2)
                            nc.vector.tensor_tensor(esum[:], esh[0][:], esh[1][:],
                                                    ALU.add)
                            rec = pB.tile([128, 1], F32, tag="rec", name="rec", bufs=2)
                            nc.vector.reciprocal(rec[:], esum[:])
                            at = pB.tile([128, P], BF16, tag=f"attr{o}",
                                         name=f"attr{o}")
                            nc.vector.tensor_scalar_mul(at[:], expv[:], rec[:])
                            att_r.append(at)

                        # [bsum|wsum][q] = att^T [v_b | rowsum(v_w)]
                        for qt in range(KP):
                            ps = psB.tile([128, 2], F32, tag="bsp",
                                          name="bsp", bufs=1)
                            for pt in range(KP):
                                nc.tensor.matmul(
                                    ps[:],
                                    att_r[pt][:, qt * 128:(qt + 1) * 128],
                                    vb_r[pt][:], start=(pt == 0),
                                    stop=(pt == KP - 1),
                                )
                            nc.vector.tensor_copy(bsum_t[qt][:], ps[:, 0:1])
                            nc.vector.tensor_copy(wsum_t[qt][:], ps[:, 1:2])
                        # W_effT[c, q] = sum_p v_w[p, c] att[p, q]
                        for qh in range(2):
                            qs = slice(qh * 512, (qh + 1) * 512)
                            for ct in range(KP):
                                ps = psB.tile([128, 512], F32, tag="wfp",
                                              name="wfp", bufs=2)
                                for pt in range(KP):
                                    nc.tensor.matmul(
                                        ps[:],
                                        v_w_r[pt][
                                            :, ct * 128:(ct + 1) * 128],
                                        att_r[pt][:, qs],
                                        start=(pt == 0),
                                        stop=(pt == KP - 1),
                                    )
                                nc.vector.tensor_copy(weff[ct][:, qs],
                                                      ps[:])

                # ============ PHASE C ============
                with (
                    tc.tile_pool(name="pC", bufs=1) as pC,
                    tc.tile_pool(name="psC", bufs=1, space="PSUM") as psC,
                    phase_loop('C'),
                ):
                    prev = None

                    def emit_mlp(h2p, x1p, csp):
                        mgp = []
                        for t in range(KP2):
                            g = pC.tile([128, 2, CH], FP8, tag=f"mg{t}",
                                        name=f"mg{t}", bufs=2)
                            mgp.append(g)
                        for j in range(KP):
                            ps = psC.tile([128, CH], F32, tag="m1",
                                          name="m1_ps", bufs=2)
                            for ct in range(KP):
                                nc.tensor.matmul(
                                    ps[:], m1w[ct][:, j * 128:(j + 1) * 128],
                                    h2p[ct][:], start=(ct == 0),
                                    stop=(ct == KP - 1),
                                )
                            nc.scalar.activation(mgp[j // 2][:, j % 2, :],
                                                 ps[:], AF.Gelu,
                                                 bias=m1b_t[j][:])
                        for o in range(KP):
                            ps = psC.tile([128, CH], F32, tag="m2",
                                          name="m2_ps", bufs=2)
                            for t in range(KP2):
                                nc.tensor.matmul(
                                    ps[:],
                                    m2p[t][:, :, o * 128:(o + 1) * 128],
                                    mgp[t][:], start=(t == 0),
                                    stop=(t == KP2 - 1), perf_mode=DR,
                                )
                            mo = pC.tile([128, CH], F32, tag="mo", name="mo",
                                         bufs=3)
                            nc.vector.scalar_tensor_tensor(
                                mo[:], ps[:], cm2_t[:], x1p[o][:],
                                op0=ALU.mult, op1=ALU.add,
                            )
                            nc.sync.dma_start(
                                outT_d[o * 128:(o + 1) * 128, csp], mo[:])

                    for c in range(NCH):
                        cs = slice(c * CH, (c + 1) * CH)
                        xt = []
                        for p in range(KP):
                            t = pC.tile([128, CH], BF16, tag=f"cxt{p}",
                                        name=f"cxt{p}", bufs=2)
                            nc.sync.dma_start(
                                t[:], xTb_d[p * 128:(p + 1) * 128, cs])
                            xt.append(t)
                        rb = pC.tile([128, CH], F32, tag="rb", name="rb",
                                     bufs=2)
                        nmb = pC.tile([128, CH], F32, tag="nmb", name="nmb",
                                      bufs=2)
                        for drow, bt in ((rho_row_d, rb), (nmr_row_d, nmb)):
                            r = drow[c * CH:(c + 1) * CH]
                            nc.scalar.dma_start(
                                bt[:],
                                bass.AP(tensor=r.tensor, offset=r.offset,
                                        ap=[[0, 128], [1, CH]]))
                        # xs = x * rho (bf16), moving operand of the apply
                        # GEMM. On GPSIMD: keeps it off the busy DVE queue so
                        # the next chunk's apply MMs aren't gated on DVE.
                        xs = []
                        for p in range(KP):
                            t = pC.tile([128, CH], BF16, tag=f"xs{p}",
                                        name=f"xs{p}", bufs=2)
                            nc.gpsimd.tensor_mul(t[:], xt[p][:], rb[:])
                            xs.append(t)
                        # apply GEMM + x1 epilogue + fp8 pair copies
                        x1, x1pair, sqpair = [], [], []
                        for t in range(KP2):
                            x1pair.append(pC.tile([128, 2, CH], FP8,
                                                  tag=f"x1p{t}",
                                                  name=f"x1p{t}", bufs=2))
                            sqpair.append(pC.tile([128, 2, CH], FP8,
                                                  tag=f"sqp{t}",
                                                  name=f"sqp{t}", bufs=2))
                        for o in range(KP):
                            ps = psC.tile([128, CH], F32, tag="tout",
                                          name="tout_ps", bufs=2)
                            for ct in range(KP):
                                nc.tensor.matmul(
                                    ps[:], weff[ct][:, o * 128:(o + 1) * 128],
                                    xs[ct][:],
                                    start=(ct == 0), stop=(ct == KP - 1),
                                )
                            ta = pC.tile([128, CH], BF16, tag="ta", name="ta",
                                         bufs=2)
                            nc.vector.scalar_tensor_tensor(
                                ta[:], nmb[:], wsum_t[o][:], xt[o][:],
                                op0=ALU.mult, op1=ALU.add,
                            )
                            xo = pC.tile([128, CH], BF16, tag=f"x1_{o}",
                                         name=f"x1_{o}", bufs=2)
                            nc.vector.scalar_tensor_tensor(
                                xo[:], ps[:], bsum_t[o][:], ta[:],
                                op0=ALU.add, op1=ALU.add,
                            )
                            x1.append(xo)
                            nc.scalar.activation(
                                x1pair[o // 2][:, o % 2, :], xo[:], AF.Copy)
                            nc.scalar.activation(
                                sqpair[o // 2][:, o % 2, :], xo[:], AF.Square)
                        # prev-chunk MLP fills PE while ACT/DVE finish pairs
                        if prev is not None:
                            emit_mlp(*prev)
                        # LN2 stats via fp8-DR ones matmuls
                        sum2 = psC.tile([128, CH], F32, tag="s2", name="s2")
                        for t in range(KP2):
                            nc.tensor.matmul(sum2[:], ones8[:], x1pair[t][:],
                                             start=(t == 0),
                                             stop=(t == KP2 - 1),
                                             perf_mode=DR)
                        msq2 = psC.tile([128, CH], F32, tag="q2", name="q2")
                        for t in range(KP2):
                            nc.tensor.matmul(msq2[:], ones8[:], sqpair[t][:],
                                             start=(t == 0),
                                             stop=(t == KP2 - 1),
                                             perf_mode=DR)
                        mu2b = pC.tile([128, CH], BF16, tag="mu2b",
                                       name="mu2b", bufs=1)
                        nc.vector.tensor_scalar(
                            mu2b[:], sum2[:], scalar1=C_STAT, scalar2=None,
                            op0=ALU.mult)
                        ms2 = pC.tile([128, CH], F32, tag="ms2", name="ms2",
                                      bufs=1)
                        nc.vector.tensor_scalar(
                            ms2[:], msq2[:], scalar1=C_STAT, scalar2=None,
                            op0=ALU.mult)
                        tv = pC.tile([128, CH], F32, tag="tv", name="tv",
                                     bufs=1)
                        nc.vector.tensor_mul(tv[:], mu2b[:], mu2b[:])
                        vr = pC.tile([128, CH], F32, tag="vr", name="vr",
                                     bufs=1)
                        nc.vector.tensor_tensor(vr[:], ms2[:], tv[:],
                                                ALU.subtract)
                        sd = pC.tile([128, CH], F32, tag="sd", name="sd",
                                     bufs=1)
                        nc.scalar.activation(sd[:], vr[:], AF.Sqrt,
                                             bias=eps_t[:])
                        rho2 = pC.tile([128, CH], BF16, tag="rho2",
                                       name="rho2", bufs=1)
                        nc.vector.reciprocal(rho2[:], sd[:])
                        h2 = []
                        for ct in range(KP):
                            ht = pC.tile([128, CH], BF16, tag="h2t",
                                         name="h2t", bufs=2)
                            nc.vector.tensor_tensor(ht[:], x1[ct][:], mu2b[:],
                                                    ALU.subtract)
                            h = pC.tile([128, CH], BF16, tag=f"h2_{ct}",
                                        name=f"h2_{ct}", bufs=2)
                            nc.vector.tensor_mul(h[:], ht[:], rho2[:])
                            h2.append(h)
                        prev = (h2, x1, cs)
                    emit_mlp(*prev)

            _loop_ctx.close()

    nc.compile()
    return nc


def _phase_a(nc, tc, bass, xT_d, qk_ext, qksum_bc, qkb_bc,
             ones_r, eps_t, iden, rho_t, nmr_t, mu_t, rhoT, nmrT,
             rho_row_d, nmr_row_d, e_ps, wload):
    with (
        tc.tile_pool(name="pA", bufs=1) as pA,
        tc.tile_pool(name="psA", bufs=1, space="PSUM") as psA,
    ):
        prev_xqc = None

        def emit_energy(xqc, cc):
            for ts in range(TS):
                for qh in range(2):
                    nc.tensor.matmul(
                        e_ps[qh][:],
                        xqc[ts][:, qh * 128:(qh + 1) * 128],
                        xqc[ts][:],
                        start=(cc == 0 and ts == 0),
                        stop=(cc == NCH - 1 and ts == TS - 1),
                        skip_group_check=True,
                    )

        for c in range(NCH):
            cs = slice(c * CH, (c + 1) * CH)
            xt = []
            for p in range(KP):
                t = pA.tile([128, CH], F32R, tag=f"xt{p}", name=f"xt{p}",
                            bufs=3)
                nc.sync.dma_start(t[:], xT_d[p * 128:(p + 1) * 128, cs])
                xt.append(t)
            # msq directly in token-part layout: tiny column matmuls
            # (stationary = x^2 sub-tile, moving = ones/P column)
            msq_ps = psA.tile([128, CH], F32, tag="msq", name="msq_ps",
                              bufs=1)
            for p in range(KP):
                sq = pA.tile([128, CH], F32R, tag=f"sq{p % 2}",
                             name=f"sq{p % 2}", bufs=2)
                nc.scalar.activation(sq[:], xt[p][:], AF.Square)
                nc.tensor.matmul(msq_ps[:], ones_r[:], sq[:],
                                 start=(p == 0), stop=(p == KP - 1))
            # transpose 128-wide slices of the broadcast tile; column 0 of
            # each transpose is the per-partition stats column
            msq_sb = pA.tile([128, CH], F32, tag="msqsb", name="msq_sb",
                             bufs=2)
            nc.vector.tensor_copy(msq_sb[:], msq_ps[:])
            msq_tp = pA.tile([128, TS], F32, tag="msqt4", name="msq_tp",
                             bufs=2)
            for ts in range(TS):
                tpo = psA.tile([128, 128], F32, tag="tp", name="tp", bufs=1)
                nc.tensor.transpose(tpo[:],
                                    msq_sb[:, ts * 128:(ts + 1) * 128],
                                    iden[:])
                nc.vector.tensor_copy(msq_tp[:, ts:ts + 1], tpo[:, 0:1])
            # x_q (token-part) + mean ride-along column
            xq_psl = []
            for ts in range(TS):
                xq_ps = psA.tile([128, P4 + 2], F32, tag=f"xq{ts % 2}",
                                 name=f"xq{ts % 2}", bufs=2)
                for p in range(KP):
                    nc.tensor.matmul(
                        xq_ps[:],
                        xt[p][:, ts * 128:(ts + 1) * 128],
                        qk_ext[p][:],
                        start=(p == 0), stop=(p == KP - 1),
                    )
                col = c * TS + ts
                nc.vector.tensor_copy(mu_t[:, col:col + 1],
                                      xq_ps[:, P4:P4 + 1])
                xq_psl.append(xq_ps)
            # rho / -mu*rho for the whole chunk in one [128,4] chain
            csl = slice(c * TS, (c + 1) * TS)
            tmp4 = pA.tile([128, TS], F32, tag="tmp4", name="tmp4", bufs=2)
            nc.vector.tensor_mul(tmp4[:], mu_t[:, csl], mu_t[:, csl])
            var4 = pA.tile([128, TS], F32, tag="var4", name="var4", bufs=2)
            nc.vector.tensor_tensor(var4[:], msq_tp[:], tmp4[:],
                                    ALU.subtract)
            sd4 = pA.tile([128, TS], F32, tag="sd4", name="sd4", bufs=2)
            nc.scalar.activation(sd4[:], var4[:], AF.Sqrt, bias=eps_t[:])
            nc.vector.reciprocal(rho_t[:, csl], sd4[:])
            tmp4b = pA.tile([128, TS], F32, tag="tmp4b", name="tmp4b", bufs=2)
            nc.vector.tensor_mul(tmp4b[:], mu_t[:, csl], rho_t[:, csl])
            nc.vector.tensor_scalar_mul(nmr_t[:, csl], tmp4b[:], -1.0)
            # correct x_q (reads PSUM directly)
            xq_c = []
            for ts in range(TS):
                col = c * TS + ts
                qtmp = pA.tile([128, P4], F32, tag="qtmp", name="qtmp",
                               bufs=2)
                nc.vector.scalar_tensor_tensor(
                    qtmp[:], qksum_bc[:], nmr_t[:, col:col + 1], qkb_bc[:],
                    op0=ALU.mult, op1=ALU.add,
                )
                xc = pA.tile([128, P4], F32R, tag=f"xqc{ts}", name=f"xqc{ts}",
                             bufs=2)
                nc.vector.scalar_tensor_tensor(
                    xc[:], xq_psl[ts][:, :P4], rho_t[:, col:col + 1], qtmp[:],
                    op0=ALU.mult, op1=ALU.add,
                )
                xq_c.append(xc)
            if prev_xqc is not None:
                emit_energy(prev_xqc, c - 1)
            prev_xqc = xq_c
            nw = (len(wload) + NCH - 1) // NCH
            for dst, src in wload[c * nw:(c + 1) * nw]:
                nc.scalar.dma_start(dst, src)
        emit_energy(prev_xqc, NCH - 1)
        # token-part [128,32] -> [32,128] row-major via PE transpose,
        # then to DRAM rows (32 contiguous 512B descriptors)
        for src, dst, drow in ((rho_t, rhoT, rho_row_d),
                               (nmr_t, nmrT, nmr_row_d)):
            tp = psA.tile([32, 128], F32, tag="tp", name="rT")
            nc.tensor.transpose(tp[:], src[:], iden[:])
            nc.vector.tensor_copy(dst[:], tp[:])
            d = drow[:]
            nc.sync.dma_start(
                bass.AP(tensor=d.tensor, offset=d.offset,
                        ap=[[128, 32], [1, 128]]),
                dst[:],
            )


_CACHE = {}


def _get_nc(loop_R=1, loop_phase='all'):
    key = (loop_R, loop_phase)
    if key not in _CACHE:
        _CACHE[key] = _build(loop_R, loop_phase)
    return _CACHE[key]


def _prep_base(inputs):
    f32 = lambda k: np.asarray(inputs[k], np.float32)
    bf16 = ml_dtypes.bfloat16
    fp8 = ml_dtypes.float8_e4m3fn
    g1, b1 = f32("ln1_g"), f32("ln1_b")
    g2, b2 = f32("ln2_g"), f32("ln2_b")
    qk_w = f32("qk_w") * g1[None, :]
    v_w = f32("v_w") * g1[None, :]
    m1_w = f32("m1_w") * g2[None, :]
    # m2 weights: fp8 pairs at x64, layout [t*128+p, i*P+o] with
    # value = m2_wT[256t+128i+p, o] * 64
    m2_wT = np.ascontiguousarray(f32("m2_w").T)          # [j, o]
    m2s = np.clip(m2_wT * S_W, -240, 240).astype(fp8)
    m2pair = m2s.reshape(KP2, 2, 128, P).transpose(0, 2, 1, 3)  # [t,p,i,o]
    return {
        "qk_wT": np.ascontiguousarray(np.concatenate(
            [qk_w.T, np.full((P, 1), 1.0 / P, np.float32),
             np.zeros((P, 1), np.float32)], axis=1)),
        "qksum": np.ascontiguousarray(qk_w.sum(axis=1)),
        "qkb": np.ascontiguousarray(f32("qk_w") @ b1),
        "v_w": np.ascontiguousarray(v_w).astype(bf16),
        "vbr": np.ascontiguousarray(np.stack([
            f32("v_b") + f32("v_w") @ b1,
            np.ascontiguousarray(v_w).astype(bf16).astype(np.float32)
            .sum(axis=1)], axis=1)).astype(bf16),
        "t1_wT": np.ascontiguousarray(f32("t1_w").T),
        "t1_b": np.ascontiguousarray(f32("t1_b")),
        "t2_wT": np.ascontiguousarray(f32("t2_w").T),
        "m1_wT": np.ascontiguousarray(m1_w.T).astype(bf16),
        "m1_b": np.ascontiguousarray(f32("m1_b") + f32("m1_w") @ b2),
        "m2p": np.ascontiguousarray(m2pair.reshape(KP2 * 128, 2 * P)),
        "ones8": np.full((128, 256), ONES8, dtype=fp8),
        "iden": np.eye(128, dtype=np.float32),
    }


def kernel(**inputs):
    return _kernel_impl(inputs, loop_R=1)


def _kernel_impl(inputs, loop_R=1, loop_phase='all'):
    x = np.ascontiguousarray(np.asarray(inputs["x"], np.float32))
    assert x.shape == (B, N, P), x.shape
    nc = _get_nc(loop_R, loop_phase)
    base = _prep_base(inputs)
    m2_b = np.asarray(inputs["m2_b"], np.float32)
    in_maps = []
    for b in range(B):
        m = dict(base)
        xT = np.ascontiguousarray(x[b].T)
        m["xT"] = xT
        m["xTb"] = xT.astype(ml_dtypes.bfloat16)
        in_maps.append(m)
    res = bass_utils.run_bass_kernel_spmd(nc, in_maps, core_ids=list(range(B)))
    out = np.empty((B, N, P), np.float32)
    for b in range(B):
        out[b] = res.results[b]["outT"].T
    out += m2_b[None, None, :]
    return out


if __name__ == "__main__":
    import sys
    import time

    sys.path.insert(0, "/root/problem")
    import reference as refmod

    inputs = {k: np.asarray(v) for k, v in refmod.setup_inputs().items()}
    t0 = time.time()
    got = kernel(**inputs)
    print(f"kernel() took {time.time() - t0:.1f}s (incl compile)")
    exp = np.asarray(refmod.reference(**inputs))
    err = np.abs(got - exp)
    l2 = np.linalg.norm(got - exp) / np.linalg.norm(exp)
    print(f"absmax={err.max():.3e} L2rel={l2:.3e}")
